# revision 12
# baseline (speedup 1.0000x reference)
"""Trainium2 Bass kernel for nn_DL_SOTA_PrototypeNet (vq_codebook), v2.

Math (all folding host-side in f64, biases are zero so c-terms vanish):
  g = gelu(x @ w1)                                  [n, 64]
  Since 1^T Wbar = 0, both per-token quadratic forms live on 1-perp:
    varh = g M g^T      (M = I/H - 11^T/H^2, LN variance)
    |u|^2 = g Ghat g^T  (Ghat = Wbar Wbar^T, u = g @ Wbar = z/r)
  Generalized eigenbasis B [64, 63] simultaneously diagonalizes both:
    w = g @ B  =>  varh = sum_j w_j^2,   |u|^2 = sum_j lam_j w_j^2
  logits L = r * (g @ Wp), r = rsqrt(varh + eps), z2 = r^2 * |u|^2.

Device pipeline per core (4 batches x 8192 tokens):
  mm1:  w1 stationary [128, 64]   -> h psum, ACT gelu -> g f16
  tail: [B |pad| Wp] stationary [64, 80] -> psum [80, 512]: w rows
        0..63, L' rows 64..70; single evac (ACT/DVE split) -> wt f16
  square: DVE f16 in-place on w rows (2x mode)
  pass3: [lam | 1]-sparse stationary [63, 16], accumulating matmul over
         w^2 chunks -> zps [16, 512] per super: (z2raw, varh) rows
  transposes (DMA xbar): wt[64:80] -> ntok (L'), zs -> ztok
  token-major: r = Rsqrt(varh+eps), softmax(L*r/T), weighted stats
  out: packed per-partition partials [128, 48]; host reduces + p2.
"""
import sys
from contextlib import ExitStack

sys.path.insert(0, "/opt/trn_rl_repo")

import numpy as np

import concourse.bass as bass
import concourse.mybir as mybir
import concourse.tile as tile
from concourse.vector_clock import ScopedClock, VectorClock

# ---------------------------------------------------------------------------
# Workaround: this walrus build only accepts 1 sync-wait per CTRL (Drain)
# instruction; Tile's tail drain carries one wait per active proc. Split it.
_orig_drain_and_barrier = tile.TileContext._drain_and_barrier


def _patched_drain_and_barrier(self, tick_clock, wait_clock):
    gclock = tick_clock.global_clock
    nprocs = len(gclock)
    procs = [i for i in range(nprocs) if gclock[i] > 0]
    for p in procs:
        vec = [gclock[i] if i == p else 0 for i in range(nprocs)]
        drain_inst = self.nc.sync.drain()
        wait_clock.add_sem_waits(drain_inst.ins, ScopedClock({None: VectorClock(vec)}))
    if not procs:
        self.nc.sync.drain()
    self.nc.all_engine_barrier()
    assert self.sems is not None
    popped = self.nc._tile_sem_poison_stack.pop()
    assert popped is self._sem_poison
    self.nc.clear_and_free_semaphores(list(self.sems.allocated().values()))
    self.nc.all_engine_barrier()


tile.TileContext._drain_and_barrier = _patched_drain_and_barrier


def _split_excess_waits(nc, max_waits=1):
    """This walrus rejects instructions with more than ~1 sync wait. Hoist
    excess waits onto same-engine NoOps placed immediately before the
    instruction (engine streams execute in order, and DMA issue happens at
    NX-execution time, so semantics are preserved)."""
    idx = 0
    for bbname, bbh in nc.bb_map.items():
        insts = bbh.bb.instructions
        out = []
        for inst in insts:
            si = getattr(inst, "sync_info", None)
            waits = list(si.on_wait) if si is not None and si.on_wait else []
            if len(waits) > max_waits:
                extra, keep = waits[:-max_waits], waits[-max_waits:]
                for w in extra:
                    nop = mybir.InstNoOp(name=f"I-waitsplit-{idx}", ins=[], outs=[])
                    idx += 1
                    nop.engine = inst.engine
                    nop.sync_info = mybir.SyncInfo(on_wait=[w], on_update=[])
                    nc.register_instruction(nop, overwrite=True)
                    out.append(nop)
                si.on_wait = keep
            out.append(inst)
        insts[:] = out
# ---------------------------------------------------------------------------

B, N, PULSE = 32, 8192, 128
H, D, K = 64, 256, 6
TEMP, LN_EPS = 0.1, 1e-5
NCORES = 8
BPC = B // NCORES              # batches per core = 4
T = BPC * N                    # tokens per core = 32768
SUPER = 4096                   # tokens per pipeline super-chunk
NSUP = T // SUPER              # supers per core = 8
SUPB = N // SUPER              # supers per batch = 2
TC = 512                       # tail chunk / matmul moving width
NTC = SUPER // TC              # tail chunks per super = 8
MC = 1024                      # mm1 psum chunk width
W = H - 1                      # w features = 63
LP0 = 64                       # first logit row (after w rows + 1 pad)
NTAIL = 80                     # tail stationary cols: 63 w | pad | 6 L' | pad
NWIN = 16                      # transpose window rows = 64..80
WIN0 = LP0                     # window start row
SLOTS = SUPER // 128           # slots per super = 32

F16 = mybir.dt.float16
F32 = mybir.dt.float32
AF = mybir.ActivationFunctionType
OP = mybir.AluOpType
AX = mybir.AxisListType


def _host_fold(w1, b1, ln_g, ln_b, w2, b2, prot):
    f64 = np.float64
    A = ln_g.astype(f64)[:, None] * w2.astype(f64)
    a_row = ln_g.astype(f64) @ w2.astype(f64)
    c_row = ln_b.astype(f64) @ w2.astype(f64) + b2.astype(f64)
    Wbar = A - np.ones((H, 1), f64) / H * a_row[None, :]
    Wp = Wbar @ prot.T.astype(f64)            # [H, K]
    cp = c_row @ prot.T.astype(f64)           # [K]
    cc = float(c_row @ c_row)
    Ghat = Wbar @ Wbar.T
    M = np.eye(H, dtype=f64) / H - np.ones((H, H), f64) / H**2
    # U: orthonormal basis of 1-perp
    v = np.ones((H, 1), f64) / np.sqrt(H)
    Qf, _ = np.linalg.qr(np.concatenate([v, np.eye(H, dtype=f64)[:, :H - 1]], 1))
    U = Qf[:, 1:]
    Mt = U.T @ M @ U
    Gt = U.T @ Ghat @ U
    dM, VM = np.linalg.eigh(Mt)
    assert dM.min() > 1e-9
    Mh = VM @ np.diag(np.sqrt(dM)) @ VM.T
    Mhi = VM @ np.diag(1.0 / np.sqrt(dM)) @ VM.T
    lam, O = np.linalg.eigh(Mhi @ Gt @ Mhi)
    lam = np.maximum(lam, 0.0)
    Bm = U @ Mh @ O                            # [H, W]
    assert abs(Bm @ Bm.T - M).max() < 1e-10
    assert abs(Bm @ np.diag(lam) @ Bm.T - Ghat).max() < 1e-8
    S2 = np.zeros((H, NTAIL), f64)             # tail stationary
    S2[:, 0:W] = Bm
    S2[:, LP0:LP0 + K] = Wp
    # pass3 stationary: per-(parity, chunk) [W, 32] slices, nonzero only
    # at cols (16q + 2i, 16q + 2i + 1): both supers of a batch accumulate
    # into one [32, 512] psum tile, evacuated once per batch.
    S3 = np.zeros((W, 2 * NTC * 32), f64)
    for q in range(2):
        for i in range(NTC):
            c0 = (q * NTC + i) * 32
            S3[:, c0 + 16 * q + 2 * i] = lam
            S3[:, c0 + 16 * q + 2 * i + 1] = 1.0
    p2 = np.sum(prot.astype(f64) ** 2, axis=1)
    return S2, S3, cp, cc, p2


OPTS = dict(
    evac_pat="dadaddda",  # per-chunk evac engine: a=ACT d=DVE (psum: no Pool)
    zevac_engine="a",     # z-psum evac engine (a/d)
    sq_pool_frac=0.375,   # fraction of the w-square columns on Pool
    sq_splits=1,          # split the per-super square into n DVE ops
    sq_inline=False,      # emit square chunks inside the tail-evac loop
    mm1_mc=1024,          # mm1 psum chunk width
    mm1_bufs=2,
    xbufs=3, gbufs=3, wbufs=4, ntbufs=2, ztbufs=2,
    p3_lag=2,             # supers of lag before a super's pass3 matmuls
    dma_split=1,          # pieces per xt input DMA
    dma_split0=2,         # pieces for the first super (startup latency)
    dma_dedicate=False,   # xt on sync queue, transposes/outputs on scalar
    p3_at_mm1=True,       # start lagged pass3 after next super's mm1 block
    tok_stride=3,         # advance token-major gens every Nth pump
    tailbufs=3, zpsbufs=1,
    xbar_engine="sync",
    # token-major engine assignment: d=DVE, p=Pool. Pool only supports
    # plain TensorTensor/TensorCopy (no ScalarPtr ops, no free-axis
    # reduces, no PSUM) — so Et/Dt/mx10/mx/sme/cnt/d2 must stay DVE.
    eng_Lt="p", eng_mx="d", eng_Et="d", eng_sme="d", eng_At="p",
    eng_Dt="d", eng_DtA="p", eng_cnt="d", eng_d2="d",
    eng_r2="p", eng_z2t="p", eng_mx10="d", eng_acc="p",
    strands=1,            # extra slot-splits per super in token-major
    strands_last=1,       # same, for the final batch (exposed tail)
)


def _build_program(num_cores, opts=None):
    o = dict(OPTS)
    if opts:
        o.update(opts)
    nc = bass.Bass("TRN2", target_bir_lowering=False, debug=False,
                   num_devices=num_cores)
    _eps_t = nc.alloc_sbuf_tensor("const-f32-eps", [128, 1], F32)
    nc.gpsimd.memset(_eps_t.ap(), LN_EPS)
    nc.const_aps.aps[(F32, LN_EPS)] = _eps_t.ap()
    nc.all_engine_barrier()
    xt = nc.dram_tensor("xt", [128, T], F16, kind="ExternalInput").ap()
    w1d = nc.dram_tensor("w1d", [128, H], F16, kind="ExternalInput").ap()
    s2d = nc.dram_tensor("s2d", [H, NTAIL], F16, kind="ExternalInput").ap()
    s3d = nc.dram_tensor("s3d", [W, 2 * NTC * 32], F16,
                         kind="ExternalInput").ap()
    b1d = nc.dram_tensor("b1d", [H, 1], F32, kind="ExternalInput").ap()
    outd = nc.dram_tensor("outd", [128, BPC * 2 * K], F32,
                          kind="ExternalOutput").ap()

    with tile.TileContext(nc) as tc, ExitStack() as ctx:
        cpool = ctx.enter_context(tc.tile_pool(name="consts", bufs=1))
        xpool = ctx.enter_context(tc.tile_pool(name="xin", bufs=o["xbufs"]))
        mm1ps = ctx.enter_context(
            tc.tile_pool(name="mm1ps", bufs=o["mm1_bufs"], space="PSUM"))
        tailps = ctx.enter_context(
            tc.tile_pool(name="tailps", bufs=o["tailbufs"], space="PSUM"))
        zps = ctx.enter_context(
            tc.tile_pool(name="zps", bufs=o["zpsbufs"], space="PSUM"))
        gpool = ctx.enter_context(tc.tile_pool(name="gtile", bufs=o["gbufs"]))
        wpool = ctx.enter_context(tc.tile_pool(name="wtile", bufs=o["wbufs"]))
        zspool = ctx.enter_context(tc.tile_pool(name="zstage", bufs=2))
        ntpool = ctx.enter_context(tc.tile_pool(name="ntok", bufs=o["ntbufs"]))
        ztpool = ctx.enter_context(tc.tile_pool(name="ztok", bufs=o["ztbufs"]))
        nstr_max = max(o["strands"], o["strands_last"]) * SUPB
        twide = ctx.enter_context(
            tc.tile_pool(name="twide", bufs=nstr_max + 2))
        spool = ctx.enter_context(
            tc.tile_pool(name="small", bufs=nstr_max + 4))
        opool = ctx.enter_context(tc.tile_pool(name="outs", bufs=2))

        w1sb = cpool.tile([128, H], F16, tag="w1sb")
        nc.sync.dma_start(w1sb[:], w1d[:])
        s2sb = cpool.tile([H, NTAIL], F16, tag="s2sb")
        nc.scalar.dma_start(s2sb[:], s2d[:])
        s3sb = cpool.tile([W, 2 * NTC * 32], F16, tag="s3sb")
        nc.scalar.dma_start(s3sb[:], s3d[:])
        b1sb = cpool.tile([H, 1], F32, tag="b1sb")
        nc.scalar.dma_start(b1sb[:], b1d[:])
        # final output staging: all batches packed, one DMA at the end;
        # host sums the 128 partition partials
        ostage = cpool.tile([128, BPC * 2 * K], F32, tag="ostage")

        xbar_eng = (nc.scalar if o["dma_dedicate"] else
                    {"sync": nc.sync, "scalar": nc.scalar}[o["xbar_engine"]])
        tok_eng = {"d": nc.vector, "p": nc.gpsimd}

        def evac(eng_ch, dst, src):
            # GPSIMD cannot access PSUM on real hw — ACT or DVE only
            if eng_ch == "a":
                nc.scalar.copy(dst, src)
            else:
                nc.vector.tensor_copy(dst, src)

        # per-batch token-major destination tiles (ntok: L' slots; ztok)
        nt_tiles = [None] * BPC
        zt_tiles = [None] * BPC

        # background generators, pumped between trunk instructions so every
        # engine stream interleaves trunk/pass3/token-major ops finely
        # (avoids head-of-line blocking on in-order engine queues).
        # Each entry: [gen, stride] — advanced every stride-th pump call.
        bg = []
        _pump_ct = [0]

        def pump(n=1):
            for _ in range(n):
                _pump_ct[0] += 1
                for e in list(bg):
                    if _pump_ct[0] % e[1]:
                        continue
                    try:
                        next(e[0])
                    except StopIteration:
                        bg.remove(e)

        xt_tiles = {}

        def fetch_x(sg):
            if sg >= NSUP or sg in xt_tiles:
                return
            tok0 = sg * SUPER
            xt_t = xpool.tile([128, SUPER], F16, tag="xt")
            in_q = nc.sync if not o["dma_dedicate"] and sg % 2 else nc.sync
            dstep = SUPER // (o["dma_split0"] if sg == 0 else o["dma_split"])
            for d0 in range(0, SUPER, dstep):
                in_q.dma_start(xt_t[:, d0:d0 + dstep],
                               xt[:, tok0 + d0:tok0 + d0 + dstep])
            xt_tiles[sg] = xt_t

        def emit_super_trunk(sg, p3start=None):
            """Feature-major trunk for super sg; returns the wt tile.
            p3start: callback fired after the mm1 block (starts the lagged
            pass3 generator at the point where the PE has mm1 queued)."""
            b, ss = sg // SUPB, sg % SUPB
            fetch_x(sg)
            xt_t = xt_tiles.pop(sg)
            if p3start is not None and not o["p3_at_mm1"]:
                p3start()
                p3start = None
            pump()
            gt = gpool.tile([H, SUPER], F16, tag="gt")
            MC = o["mm1_mc"]
            for p0 in range(0, SUPER, MC):
                h_ps = mm1ps.tile([H, MC], F32, tag="h")
                for c0 in range(0, MC, TC):
                    nc.tensor.matmul(h_ps[:, c0:c0 + TC], w1sb[:],
                                     xt_t[:, p0 + c0:p0 + c0 + TC],
                                     start=True, stop=True)
                pump()
                nc.scalar.activation(gt[:, p0:p0 + MC], h_ps[:],
                                     AF.Gelu, bias=b1sb[:])
                pump()
            if p3start is not None:
                p3start()
            fetch_x(sg + 1)   # prefetch next super's input
            # tail matmuls + evac, with the w-square emitted per completed
            # column range so pass3 of this super can begin promptly
            wt = wpool.tile([NTAIL, SUPER], F16, tag="wt")
            nsq = o["sq_splits"]
            sqstep = NTC // nsq
            spf = o["sq_pool_frac"]
            for i in range(NTC):
                c0 = i * TC
                t_ps = tailps.tile([NTAIL, TC], F32, tag="t")
                nc.tensor.matmul(t_ps[:], s2sb[:], gt[:, c0:c0 + TC],
                                 start=True, stop=True)
                pump()
                evac(o["evac_pat"][i], wt[:, c0:c0 + TC], t_ps[:])
                pump()
                if o["sq_inline"] and (i + 1) % sqstep == 0:
                    s0, s1 = (i + 1 - sqstep) * TC, (i + 1) * TC
                    pw = int((s1 - s0) * spf) // 128 * 128
                    if pw:
                        nc.gpsimd.tensor_mul(wt[0:W, s0:s0 + pw],
                                             wt[0:W, s0:s0 + pw],
                                             wt[0:W, s0:s0 + pw])
                        pump()
                    nc.vector.tensor_mul(wt[0:W, s0 + pw:s1],
                                         wt[0:W, s0 + pw:s1],
                                         wt[0:W, s0 + pw:s1])
                    pump()
            if not o["sq_inline"]:
                for j in range(nsq):
                    s0, s1 = j * (SUPER // nsq), (j + 1) * (SUPER // nsq)
                    pw = int((s1 - s0) * spf) // 128 * 128
                    if pw:
                        nc.gpsimd.tensor_mul(wt[0:W, s0:s0 + pw],
                                             wt[0:W, s0:s0 + pw],
                                             wt[0:W, s0:s0 + pw])
                        pump()
                    nc.vector.tensor_mul(wt[0:W, s0 + pw:s1],
                                         wt[0:W, s0 + pw:s1],
                                         wt[0:W, s0 + pw:s1])
                    pump()
            # narrow transpose: rows 64..80 (6 L' + 10 garbage)
            if nt_tiles[b] is None:
                nt = ntpool.tile([128, N // 128 * NWIN], F16, tag="nt")
                nt_tiles[b] = nt
            nt3 = nt_tiles[b].rearrange("p (g c) -> p g c", c=NWIN)
            xbar_eng.dma_start_transpose(
                nt3[:, ss * SLOTS:(ss + 1) * SLOTS, :],
                wt[WIN0:NTAIL, :])
            pump()
            return wt

        zb_tiles = {}

        def pass3_gen(sg, wt, on_done):
            """lam/ones accumulating matmul over w^2 (both supers of a
            batch share one [32, TC] psum tile); z-evac + transpose once
            per batch."""
            b, q = sg // SUPB, sg % SUPB
            if q == 0:
                z_ps = zps.tile([2 * NWIN, TC], F32, tag="z")
                zb_tiles[b] = z_ps
            else:
                z_ps = zb_tiles.pop(b)
            for i in range(NTC):
                c0 = i * TC
                s0 = (q * NTC + i) * 32
                nc.tensor.matmul(z_ps[:], s3sb[:, s0:s0 + 32],
                                 wt[0:W, c0:c0 + TC],
                                 start=(q == 0 and i == 0),
                                 stop=(q == 1 and i == NTC - 1))
                if i % 2 == 1:
                    yield
            if q == 1:
                zs = zspool.tile([2 * NWIN, TC], F16, tag="zs")
                evac(o["zevac_engine"], zs[:], z_ps[:])
                yield
                if zt_tiles[b] is None:
                    zt = ztpool.tile([128, SUPB * 4 * NWIN], F16, tag="zt")
                    zt_tiles[b] = zt
                zt3 = zt_tiles[b].rearrange("p (g c) -> p g c", c=2 * NWIN)
                xbar_eng.dma_start_transpose(zt3[:], zs[:])
            on_done()

        def bcs(ap_2d, SL):
            return ap_2d.rearrange("p (g c) -> p g c", c=1).to_broadcast(
                (128, SL, K))

        def tokmajor_strand(b, ss, ia, ib, o_cnt, o_d2, first):
            """Token-major chain for super-half ss, tail-chunks ia..ib
            (slots ia*4..ib*4), as a generator yielding between ops."""
            SL = (ib - ia) * 4
            nt3 = nt_tiles[b].rearrange("p (g c) -> p g c", c=NWIN)
            sl0 = ss * SLOTS + ia * 4
            Lp3 = nt3[:, sl0:sl0 + SL, 0:K]                    # [128,SL,6] f16
            # zt layout: col = ss*64 + s4*16 + 2i + zbit
            zt5 = zt_tiles[b].rearrange(
                "p (s4 sh i two) -> p sh i s4 two", sh=SUPB, s4=4, i=NTC,
                two=2)
            nch = ib - ia
            z2raw = zt5[:, ss:ss + 1, ia:ib, :, 0:1]           # [128,1,n,4,1]
            varh = zt5[:, ss:ss + 1, ia:ib, :, 1:2]
            # max over raw logits first — runs parallel to the r chain
            # (max commutes with the positive per-token scale r)
            mx = spool.tile([128, SL], F32, tag="mx")
            tok_eng[o["eng_mx"]].tensor_reduce(mx[:], Lp3, AX.X, OP.max)
            yield
            sqv = spool.tile([128, SL], F32, tag="sqv")
            sqv4 = sqv.rearrange("p (o i s f) -> p o i s f", o=1, i=nch, s=4,
                                 f=1)
            nc.scalar.activation(sqv4[:], varh, AF.Sqrt, bias=LN_EPS)
            yield
            rv = spool.tile([128, SL], F32, tag="rv")
            nc.vector.reciprocal(rv[:], sqv[:])
            yield
            r2 = spool.tile([128, SL], F32, tag="r2")
            tok_eng[o["eng_r2"]].tensor_mul(r2[:], rv[:], rv[:])
            yield
            z2t = spool.tile([128, SL], F32, tag="z2t")
            z2t4 = z2t.rearrange("p (o i s f) -> p o i s f", o=1, i=nch, s=4,
                                 f=1)
            r24 = r2.rearrange("p (o i s f) -> p o i s f", o=1, i=nch, s=4,
                               f=1)
            tok_eng[o["eng_z2t"]].tensor_tensor(z2t4[:], r24[:], z2raw,
                                                OP.mult)
            yield
            Lt = twide.tile([128, SL * K], F16, tag="Lt")
            Lt3 = Lt.rearrange("p (g c) -> p g c", c=K)
            tok_eng[o["eng_Lt"]].tensor_tensor(Lt3[:], Lp3, bcs(rv[:], SL),
                                               OP.mult)
            yield
            mx10 = spool.tile([128, SL], F32, tag="mx10")
            tok_eng[o["eng_mx10"]].scalar_tensor_tensor(
                mx10[:], mx[:], 1.0 / TEMP, rv[:], OP.mult, OP.mult)
            yield
            Et = twide.tile([128, SL * K], F16, tag="Et")
            Et3 = Et.rearrange("p (g c) -> p g c", c=K)
            tok_eng[o["eng_Et"]].scalar_tensor_tensor(
                Et3[:], Lt3[:], 1.0 / TEMP, bcs(mx10[:], SL), OP.mult,
                OP.subtract)
            yield
            nc.scalar.activation(Et[:], Et[:], AF.Exp)
            yield
            sme = spool.tile([128, SL], F32, tag="sme")
            tok_eng[o["eng_sme"]].tensor_reduce(sme[:], Et3[:], AX.X, OP.add)
            yield
            rec = spool.tile([128, SL], F32, tag="rec")
            nc.vector.reciprocal(rec[:], sme[:])
            yield
            At = twide.tile([128, SL * K], F16, tag="At")
            At3 = At.rearrange("p (g c) -> p g c", c=K)
            tok_eng[o["eng_At"]].tensor_tensor(At3[:], Et3[:],
                                               bcs(rec[:], SL), OP.mult)
            yield
            Dt = twide.tile([128, SL * K], F16, tag="Dt")
            Dt3 = Dt.rearrange("p (g c) -> p g c", c=K)
            tok_eng[o["eng_Dt"]].scalar_tensor_tensor(
                Dt3[:], Lt3[:], -2.0, bcs(z2t[:], SL), OP.mult, OP.add)
            yield
            DtA = twide.tile([128, SL * K], F16, tag="DtA")
            tok_eng[o["eng_DtA"]].tensor_mul(DtA[:], Dt[:], At[:])
            yield
            At_r = At.rearrange("p (g c) -> p c g", c=K)
            DtA_r = DtA.rearrange("p (g c) -> p c g", c=K)
            if first:
                tok_eng[o["eng_cnt"]].tensor_reduce(o_cnt[:], At_r[:], AX.X,
                                                    OP.add)
                yield
                tok_eng[o["eng_d2"]].tensor_reduce(o_d2[:], DtA_r[:], AX.X,
                                                   OP.add)
            else:
                p_cnt = spool.tile([128, K], F32, tag="p_cnt")
                tok_eng[o["eng_cnt"]].tensor_reduce(p_cnt[:], At_r[:], AX.X,
                                                    OP.add)
                yield
                tok_eng[o["eng_acc"]].tensor_add(o_cnt[:], o_cnt[:], p_cnt[:])
                yield
                p_d2 = spool.tile([128, K], F32, tag="p_d2")
                tok_eng[o["eng_d2"]].tensor_reduce(p_d2[:], DtA_r[:], AX.X,
                                                   OP.add)
                yield
                tok_eng[o["eng_acc"]].tensor_add(o_d2[:], o_d2[:], p_d2[:])

        def tokmajor_gen(b):
            o_cnt = opool.tile([128, K], F32, tag="o_cnt")
            o_d2 = opool.tile([128, K], F32, tag="o_d2")
            nstr = o["strands_last"] if b == BPC - 1 else o["strands"]
            step = NTC // nstr
            gens = [tokmajor_strand(b, ss, i0, i0 + step, o_cnt, o_d2,
                                    ss == 0 and i0 == 0)
                    for ss in range(SUPB) for i0 in range(0, NTC, step)]
            live = list(gens)
            while live:
                nxt = []
                for g in live:
                    try:
                        next(g)
                        nxt.append(g)
                    except StopIteration:
                        pass
                live = nxt
                yield
            # pack per-batch partials into the staging tile (SBUF, Pool)
            nc.gpsimd.tensor_copy(ostage[:, (2 * b) * K:(2 * b + 1) * K],
                                  o_cnt[:])
            yield
            nc.gpsimd.tensor_copy(
                ostage[:, (2 * b + 1) * K:(2 * b + 2) * K], o_d2[:])
            nt_tiles[b] = None
            zt_tiles[b] = None

        # software pipeline across supers; pass3 lagged so the PE never
        # waits on the current super's evac+square chain; pass3 and
        # token-major run as pumped background generators.
        pend_p3 = []      # (sg, wt) awaiting pass3 start
        ready_tok = []    # batches whose z/n data is fully emitted

        def start_p3(psg, pwt):
            def on_done():
                if psg % SUPB == SUPB - 1:
                    ready_tok.append(psg // SUPB)
            bg.append([pass3_gen(psg, pwt, on_done), 1])

        for sg in range(NSUP):
            p3s = None
            if len(pend_p3) >= o["p3_lag"]:
                args = pend_p3.pop(0)
                p3s = (lambda a: lambda: start_p3(*a))(args)
            wt = emit_super_trunk(sg, p3s)
            pend_p3.append((sg, wt))
            while ready_tok:
                bg.append([tokmajor_gen(ready_tok.pop(0)), o["tok_stride"]])
        while pend_p3 or bg:
            if pend_p3:
                start_p3(*pend_p3.pop(0))
            else:
                for e in bg:
                    e[1] = 1   # trunk done: drain at full rate
            pump(4)
            while ready_tok:
                bg.append([tokmajor_gen(ready_tok.pop(0)), o["tok_stride"]])
        nc.sync.dma_start(outd[:], ostage[:])

    _split_excess_waits(nc)
    return nc


def kernel(x, w1, b1, ln_g, ln_b, w2, b2, prototypes):
    x = np.asarray(x, dtype=np.float32)
    w1 = np.asarray(w1, dtype=np.float32)
    b1 = np.asarray(b1, dtype=np.float32)
    ln_g = np.asarray(ln_g, dtype=np.float32)
    ln_b = np.asarray(ln_b, dtype=np.float32)
    w2 = np.asarray(w2, dtype=np.float32)
    b2 = np.asarray(b2, dtype=np.float32)
    prot = np.asarray(prototypes, dtype=np.float32)

    S2, S3, cp, cc, p2 = _host_fold(w1, b1, ln_g, ln_b, w2, b2, prot)
    if max(abs(cp).max(), abs(cc)) > 1e-12:
        raise NotImplementedError(
            "nonzero ln_b/b2 path not emitted (inputs have zero bias)")

    s2_np = S2.astype(np.float16)
    s3_np = S3.astype(np.float16)
    w1_np = w1.astype(np.float16)
    b1_np = b1.reshape(H, 1).astype(np.float32)

    from concourse.bass_utils import run_bass_kernel_spmd

    nc = _build_program(NCORES)
    in_maps = []
    for c in range(NCORES):
        xs = x[c * BPC:(c + 1) * BPC].reshape(T, PULSE)
        xt_np = np.ascontiguousarray(xs.T).astype(np.float16)
        in_maps.append({"xt": xt_np, "w1d": w1_np, "s2d": s2_np,
                        "s3d": s3_np, "b1d": b1_np})

    res = run_bass_kernel_spmd(nc, in_maps, core_ids=list(range(NCORES)))

    var = np.empty((B, K), np.float32)
    for c in range(NCORES):
        o = res.results[c]["outd"].astype(np.float64)   # [128, BPC*2*K]
        o = o.sum(axis=0).reshape(BPC, 2, K)
        C0 = o[:, 0]                                    # [BPC, K]
        Dsum = o[:, 1]                                  # [BPC, K]
        cnt = C0 + 1e-6
        v = Dsum / cnt + p2[None, :] * C0 / cnt
        var[c * BPC:(c + 1) * BPC] = v.astype(np.float32)
    return var
